# revision 16
# baseline (speedup 1.0000x reference)
"""ConvTranspose3d(64->32, k=3, stride=2, pad=1, out_pad=1, dilation=2) on 8 NeuronCores.

Math: with stride=2, dilation=2, padding=1, k=3, every populated output
position o = 2j+1 is odd in all three spatial dims, so the transposed conv
collapses to a dense 3^3 conv y = conv3d(x, wc, padding=1) on the 32^3 grid
(wc = flip(transpose(w))), scattered into the odd sub-lattice of the 66^3
output. Every other output voxel is exactly bias and is filled host-side, so
the device only moves the 1.05MB/core of real conv data (the baseline wrote
the full 9MB/core of mostly-bias planes and was DMA-bound).

Sharding: 8 shards = 2 batches x 2 depth-halves x 2 row-halves; each core
computes 16 conv planes x 16 rows x 32 cols x 32 c_out.

Implicit GEMM with sliding depth chunks: K = (2 input planes x 64 c_in),
M = 128 = (4 slots x 32 c_out) where chunk p = planes {p, p+1} (p = 2t-1)
feeds output-plane slots {p-1, p, p+1, p+2}. Interior chunks have 6 of 8
(plane-in, slot) blocks nonzero (75% PE efficiency vs 50% for the fixed
block-Toeplitz grouping). Per chunk: 9 matmuls (one per spatial tap, rhs =
shifted slice of the padded input pair), N = 512 = 16x32 pixels, bf16 in /
fp32 PSUM. Output plane 2m+pip = chunk m slots {s2,s3} + chunk m+1 slots
{s0,s1}, combined with one fused scalar_tensor_tensor per pair:
ct = (ps[m][64:128] + bias) + ps[m+1][0:64].  81 matmuls/core total.
"""

import sys

sys.path.insert(0, "/opt/trn_rl_repo")

import numpy as np

N_CORES = 8
N_CHUNKS = 9  # sliding depth chunks, pairs {2t-1, 2t}, t = 0..8


_cache = {}


def _build_nc():
    import concourse.bass as bass
    import concourse.tile as tile
    from concourse import bacc, mybir

    dt = mybir.dt
    nc = bacc.Bacc("TRN2", target_bir_lowering=False, debug=False,
                   num_devices=N_CORES)

    # xs: 9 pairs of adjacent padded input planes (local planes 2t-1, 2t);
    # partition p = dpi*64 + ci; 18 rows (h halo) x 34 cols (w halo).
    xs = nc.dram_tensor("xs", [N_CHUNKS, 128, 18, 34], dt.bfloat16,
                        kind="ExternalInput")
    # tw: 9 spatial-tap stationary blocks; block tap=kh*3+kw, col s*32+co,
    # row dpi*64+ci, value wc[co,ci,kd,kh,kw] with kd = dpi - s + 2.
    tw = nc.dram_tensor("tw", [128, 9 * 128], dt.bfloat16,
                        kind="ExternalInput")
    bias = nc.dram_tensor("bias", [64, 1], dt.float32, kind="ExternalInput")
    # out: compact conv result; partition (m%2)*64 + pip*32 + co,
    # col (m//2, rr*32+cc) -> conv[co, 2m+pip, rr, cc].
    out = nc.dram_tensor("out", [128, 4, 512], dt.float32,
                         kind="ExternalOutput")

    with tile.TileContext(nc) as tc:
        with (
            tc.tile_pool(name="tw", bufs=1) as tw_pool,
            tc.tile_pool(name="xp", bufs=1) as xp_pool,
            tc.tile_pool(name="bias", bufs=1) as bias_pool,
            tc.tile_pool(name="ct", bufs=1) as ct_pool,
            tc.tile_pool(name="tmp", bufs=3) as tmp_pool,
            tc.tile_pool(name="ps", bufs=4, space="PSUM") as ps_pool,
        ):
            tw_t = tw_pool.tile([128, 9 * 128], dt.bfloat16)
            bias_t = bias_pool.tile([64, 1], dt.float32)
            xp = [xp_pool.tile([128, 18, 34], dt.bfloat16, tag=f"xp{t}",
                               name=f"xp{t}")
                  for t in range(N_CHUNKS)]
            ct = ct_pool.tile([128, 4, 512], dt.float32)
            dum = ct_pool.tile([128, 512], dt.bfloat16, name="dum")

            # dma_start costs ~0.7us of descriptor generation on the issuing
            # sequencer, so spread the issues across both HW-DGE engines
            # (sync + scalar); critical deps (tw, xp0) go first on each ring.
            nc.sync.dma_start(tw_t[:, 0:3 * 128], tw[:, 0:3 * 128])
            nc.sync.dma_start(tw_t[:, 3 * 128:], tw[:, 3 * 128:])
            nc.scalar.dma_start(xp[0][:], xs[0])
            nc.scalar.dma_start(xp[1][:], xs[1])
            for t in range(2, N_CHUNKS, 2):
                nc.sync.dma_start(xp[t][:], xs[t])
            for t in range(3, N_CHUNKS, 2):
                nc.scalar.dma_start(xp[t][:], xs[t])
            nc.scalar.dma_start(bias_t[:], bias[:])

            # The PE ramps from a low DVFS p-state over its first ~5us of
            # busy time (427ns vs 216ns per 512-row matmul). Spend the
            # input-DMA wait on dummy matmuls over memset data so the clock
            # is ramped when real data lands.
            N_WARM = 14
            nc.gpsimd.memset(dum[:], 0.0)
            wps = ps_pool.tile([128, 512], dt.float32, name="wps",
                               tag="warm")
            prev_mm = None
            for w in range(N_WARM):
                mm = nc.tensor.matmul(wps[:], dum[:, 0:128], dum[:],
                                      start=True, stop=True)
                if prev_mm is not None:
                    tile.add_dep_helper(mm.ins, prev_mm.ins, sync=False,
                                        reason="warmup order")
                prev_mm = mm

            add = mybir.AluOpType.add
            ps = []
            his = []
            for t in range(N_CHUNKS):
                pst = ps_pool.tile([128, 16, 32], dt.float32)
                for tap in range(9):
                    kh, kw = tap // 3, tap % 3
                    mm = nc.tensor.matmul(
                        pst[:], tw_t[:, tap * 128:(tap + 1) * 128],
                        xp[t][:, kh:kh + 16, kw:kw + 32],
                        start=(tap == 0), stop=(tap == 8))
                    # keep the PE's static order group-contiguous so each
                    # combine fires right after its chunk's 9th matmul
                    if tap == 0 and prev_mm is not None:
                        tile.add_dep_helper(mm.ins, prev_mm.ins, sync=False,
                                            reason="group-contiguous PE order")
                prev_mm = mm
                ps.append(pst)

                # DVE can read only one PSUM operand (and GPSIMD cannot read
                # PSUM at all): stage this chunk's upper slots in SBUF via
                # the ACT engine; the copy overlaps the next chunk's matmuls.
                if t < N_CHUNKS - 1:
                    hi = tmp_pool.tile([64, 512], dt.float32, name=f"hi{t}")
                    nc.scalar.activation(
                        hi[:], pst[:].rearrange("p h w -> p (h w)")[64:128, :],
                        mybir.ActivationFunctionType.Copy)
                    his.append(hi)
                if t >= 1:
                    m = t - 1
                    p0 = (m % 2) * 64
                    c2 = m // 2
                    lo = pst[:].rearrange("p h w -> p (h w)")[0:64, :]
                    dst = ct[p0:p0 + 64, c2, :]
                    if m < 7:
                        nc.vector.scalar_tensor_tensor(
                            dst, his[m][:], bias_t[:], lo, op0=add, op1=add)
                        nc.sync.dma_start(out[p0:p0 + 64, c2, :], dst)
                    else:
                        # last pair: halve the combine so the first DMA
                        # overlaps the second half's DVE work
                        for h0 in (0, 256):
                            nc.vector.scalar_tensor_tensor(
                                dst[:, h0:h0 + 256], his[m][:, h0:h0 + 256],
                                bias_t[:], lo[:, h0:h0 + 256],
                                op0=add, op1=add)
                            nc.sync.dma_start(
                                out[p0:p0 + 64, c2, h0:h0 + 256],
                                dst[:, h0:h0 + 256])

    nc.compile()
    return nc


def _prep_shared(weight, bias):
    import ml_dtypes

    # wc[co, ci, kd, kh, kw] = weight[ci, co, 2-kd, 2-kh, 2-kw]
    wc = np.ascontiguousarray(
        weight.transpose(1, 0, 2, 3, 4)[:, :, ::-1, ::-1, ::-1])
    tw = np.zeros((128, 9, 128), np.float32)
    for dpi in range(2):
        for s in range(4):
            kd = dpi - s + 2
            if 0 <= kd <= 2:
                blk = wc[:, :, kd]  # [co, ci, kh, kw]
                tw[dpi * 64:(dpi + 1) * 64, :, s * 32:(s + 1) * 32] = \
                    blk.transpose(1, 2, 3, 0).reshape(64, 9, 32)
    tw = np.ascontiguousarray(tw.reshape(128, 9 * 128)).astype(
        ml_dtypes.bfloat16)
    bias64 = np.ascontiguousarray(
        np.tile(bias.astype(np.float32), 2).reshape(64, 1))
    return tw, bias64


def _make_slabs(x):
    """xs[core] = [9, 128, 18, 34] bf16 for core = n*4 + dh*2 + hh."""
    import ml_dtypes

    xpad = np.zeros((2, 64, 34, 34, 34), np.float32)
    xpad[:, :, 1:33, 1:33, 1:33] = x
    slabs = []
    for core in range(N_CORES):
        n, r = divmod(core, 4)
        dh, hh = divmod(r, 2)
        d0, h0 = 16 * dh, 16 * hh
        xs = np.empty((N_CHUNKS, 128, 18, 34), np.float32)
        for t in range(N_CHUNKS):
            # local planes (2t-1, 2t) -> xpad depth d0+2t, d0+2t+1
            sl = xpad[n, :, d0 + 2 * t:d0 + 2 * t + 2, h0:h0 + 18, :]
            xs[t] = sl.transpose(1, 0, 2, 3).reshape(128, 18, 34)
        slabs.append(xs.astype(ml_dtypes.bfloat16))
    return slabs


def kernel(x, weight, bias):
    from concourse.bass_utils import run_bass_kernel_spmd

    if "nc" not in _cache:
        _cache["nc"] = _build_nc()
    nc = _cache["nc"]

    x = np.asarray(x, np.float32)
    weight = np.asarray(weight, np.float32)
    bias = np.asarray(bias, np.float32)

    tw, bias64 = _prep_shared(weight, bias)
    slabs = _make_slabs(x)
    in_maps = [{"xs": slabs[core], "tw": tw, "bias": bias64}
               for core in range(N_CORES)]

    res = run_bass_kernel_spmd(nc, in_maps, core_ids=list(range(N_CORES)))

    full = np.empty((2, 32, 66, 66, 66), np.float32)
    full[...] = bias[None, :, None, None, None]
    for core in range(N_CORES):
        n, r = divmod(core, 4)
        dh, hh = divmod(r, 2)
        arr = np.asarray(res.results[core]["out"], np.float32)
        # [mhalf, pip, co, c2, rr, cc] -> [co, o=c2*4+mhalf*2+pip, rr, cc]
        conv = arr.reshape(2, 2, 32, 4, 16, 32).transpose(2, 3, 0, 1, 4, 5) \
            .reshape(32, 16, 16, 32)
        full[n, :, 32 * dh + 1:32 * dh + 33:2,
             32 * hh + 1:32 * hh + 33:2, 1:65:2] = conv
    return full
